# revision 25
# baseline (speedup 1.0000x reference)
"""BertSelfAttention (B=4, S=2048, D=768, H=12) on 8 Trainium2 NeuronCores.

Sharding: 8 cores = 4 batches x 2 head-groups (6 heads each). Per core,
for its (batch b, head-group g):

  Q^T = Wq_g^T @ x_b^T          [384, 2048]   (d_local on partitions)
  K^T = Wk_g^T @ x_b^T          [384, 2048]
  V   = x_b @ Wv_g              [2048, 384]   (seq on partitions)
  per (head h, q-half of 1024):
    S^T[k, q] = sum_dh K^T[dh, k] Q^T[dh, q]            (PE, K=64)
    P^T[k, q] = exp(S^T * 1/8 + mask[k])                (ScalarE, one
                                                         [128,1024] op/chunk)
    C^T[d, q], denom[q] = sum_k Vaug_h[k, d|1] P^T[k,q] (PE accumulate; V is
                                                         ones-augmented so the
                                                         softmax denominator is
                                                         row 64 of the output)
    out[h] = C^T[0:64] * (1/denom)                      (DVE recip; K=1 f32r
                                                         PE broadcast into rows
                                                         64:128 of the SAME ctx
                                                         PSUM tile; DVE mul)

Schedule: one pipelined region; ScalarE (the exp) is the bottleneck engine
and runs its 192 ops back to back once started.
 - All projection chains borrow the attention's double-buffered scores PSUM
   slots, so no separate projection pools (PSUM: scores 2x2 banks + ctx 2x2).
 - xT is DMA'd as 24 [128,512] column-group tiles, wv/biases first: each V
   chain needs only one column group, so the 16 V chains run entirely under
   the input DMA; the Q^T/K^T chains follow at full PE speed.
 - Attention inner loop is software-pipelined: scores(kc+2) after PV(kc),
   rolling into the next block at block end (pipeline refill), so ScalarE
   crosses block boundaries with zero idle.
 - The per-head tail (recip/broadcast/normalize) is deferred two iterations
   into the next block so its b_ps matmuls never wait on the reciprocal's
   DVE latency; the broadcast writes rows 64:128 of the ctx tile in f32r.

All big matmuls run in float32r (single-pass PE matmul: 4x the throughput of
fp32's two half-speed passes; operands are rounded to f32r on write).
Softmax skips max-subtraction: scores ~ N(0,1) here, exp is safe in fp32.
Host side only slices/transposes for layout - all FLOPs run on device.
"""

import numpy as np

import concourse.mybir as mybir
import concourse.tile as tile
from concourse import bacc
from concourse.bass_utils import run_bass_kernel_spmd

F32 = mybir.dt.float32
F32R = mybir.dt.float32r
U32 = mybir.dt.uint32
ONE_F32_BITS = 0x3F800000  # f32r memset is rejected by codegen; write raw 1.0f

N_CORES = 8
B, S, D, H = 4, 2048, 768, 12
HL = 6           # heads per core
DH = 64          # head dim
DL = HL * DH     # 384: local output dim
DCH = D // 128   # 6 contraction chunks
MCH = DL // 128  # 3 output-partition chunks for Q^T/K^T
SCH = S // 128   # 16 seq chunks
VSTRIDE = 128    # per-head stride in the augmented-V tile (aligned slices)
QW = 1024        # q-block width in the attention loop
QB = S // QW     # 2 q-blocks per head
NBLK = HL * QB   # 12 (head, q-block) attention blocks

_cached = {}


def build_program(reps=1):
    """reps>1 repeats the whole computation in one NEFF - used only by
    test.py to amortize dispatch overhead when measuring HW exec time."""
    if ("nc", reps) in _cached:
        return _cached[("nc", reps)]
    nc = bacc.Bacc("TRN2", target_bir_lowering=False, debug=False, num_devices=1)
    xT = nc.dram_tensor("xT", [D, S], F32, kind="ExternalInput").ap()
    wq = nc.dram_tensor("wq", [D, DL], F32, kind="ExternalInput").ap()
    wk = nc.dram_tensor("wk", [D, DL], F32, kind="ExternalInput").ap()
    wv = nc.dram_tensor("wv", [D, DL], F32, kind="ExternalInput").ap()
    bq = nc.dram_tensor("bq", [128, MCH], F32, kind="ExternalInput").ap()
    bk = nc.dram_tensor("bk", [128, MCH], F32, kind="ExternalInput").ap()
    bv = nc.dram_tensor("bv", [1, DL], F32, kind="ExternalInput").ap()
    mask = nc.dram_tensor("mask", [128, SCH], F32, kind="ExternalInput").ap()
    out = nc.dram_tensor("out", [HL, DH, S], F32, kind="ExternalOutput").ap()

    EXP = mybir.ActivationFunctionType.Exp
    MULT = mybir.AluOpType.mult

    blocks = [(h, qb) for h in range(HL) for qb in range(QB)]

    with tile.TileContext(nc) as tc:
      for _rep in range(reps):
        with tc.tile_pool(name="persist", bufs=1) as persist:
            QT = [persist.tile([128, S], F32R, tag=f"qt{i}", name=f"qt{i}")
                  for i in range(MCH)]
            KT = [persist.tile([128, S], F32R, tag=f"kt{i}", name=f"kt{i}")
                  for i in range(MCH)]
            V = [persist.tile([128, HL * VSTRIDE], F32R, tag=f"v{i}", name=f"v{i}")
                 for i in range(SCH)]
            mask_sb = persist.tile([128, SCH], F32, tag="mask")
            ones64 = persist.tile([1, 64], F32R, tag="ones64")
            nc.sync.dma_start(mask_sb[:], mask[:])
            nc.vector.memset(ones64[:].bitcast(U32), ONE_F32_BITS)
            for i in range(SCH):
                # presets the ones column (col 64 of each head block); only
                # those 6 strided columns - the rest is overwritten by the
                # V-projection copy
                nc.vector.memset(
                    V[i][:].rearrange("p (h j) -> p h j", j=VSTRIDE)
                    [:, :, 64:65].bitcast(U32),
                    ONE_F32_BITS)

            with tc.tile_pool(name="load", bufs=1) as load, \
                 tc.tile_pool(name="pt", bufs=3) as ptp, \
                 tc.tile_pool(name="ob", bufs=2) as obp, \
                 tc.tile_pool(name="rc", bufs=1) as rcp, \
                 tc.tile_pool(name="scps", bufs=2, space="PSUM") as scp, \
                 tc.tile_pool(name="ctxps", bufs=2, space="PSUM") as ctxp:

                # ---- input DMA. xT is loaded as 24 [128,512] column-chunk
                # tiles: each V chain needs only one 512-column group, so the
                # V projections run entirely under the input DMA. Order:
                # wv, xT col-groups, biases, wq, wk. ----
                w_sb = {nm: [None] * DCH for nm in ("q", "k", "v")}
                w_dram = {"q": wq, "k": wk, "v": wv}

                def load_w(nm, dc):
                    t = load.tile([128, DL], F32R, tag=f"w{nm}{dc}",
                                  name=f"w{nm}{dc}")
                    nc.sync.dma_start(
                        t[:], w_dram[nm][dc * 128:(dc + 1) * 128, :].bitcast(F32R))
                    w_sb[nm][dc] = t

                ones_row = load.tile([1, 128], F32R, tag="ones_row")
                nc.vector.memset(ones_row[:].bitcast(U32), ONE_F32_BITS)
                for dc in range(DCH):
                    load_w("v", dc)
                xt_sb = [[None] * DCH for _ in range(4)]  # [col j][dc]

                def load_xt_col(j):
                    for dc in range(DCH):
                        t = load.tile([128, 512], F32R, tag=f"xt{j}_{dc}",
                                      name=f"xt{j}_{dc}")
                        nc.sync.dma_start(
                            t[:],
                            xT[dc * 128:(dc + 1) * 128,
                               j * 512:(j + 1) * 512].bitcast(F32R))
                        xt_sb[j][dc] = t

                bq_sb = load.tile([128, MCH], F32, tag="bq")
                nc.sync.dma_start(bq_sb[:], bq[:])
                bk_sb = load.tile([128, MCH], F32, tag="bk")
                nc.sync.dma_start(bk_sb[:], bk[:])
                bv_sb = load.tile([1, DL], F32R, tag="bv")
                nc.sync.dma_start(bv_sb[:], bv[:].bitcast(F32R))
                for j in range(4):
                    load_xt_col(j)
                for dc in range(DCH):
                    load_w("q", dc)
                for dc in range(DCH):
                    load_w("k", dc)

                # ---- projection chains (borrowing the scores PSUM slots) ----
                def qk_chain(wname, bt, dst, mi, q):
                    ps = scp.tile([128, 512], F32, tag="s", name="ps_qk")
                    for dc in range(DCH):
                        nc.tensor.matmul(
                            ps[:],
                            w_sb[wname][dc][:, mi * 128:(mi + 1) * 128],
                            xt_sb[q][dc][:],
                            start=(dc == 0), stop=(dc == DCH - 1),
                        )
                    # bias add fused into the PSUM->SBUF copy
                    nc.vector.tensor_scalar_add(
                        dst[mi][:, q * 512:(q + 1) * 512],
                        ps[:], bt[:, mi:mi + 1],
                    )

                def v_chain(sc):
                    j, c = divmod(sc, 4)
                    ps = scp.tile([128, DL], F32, tag="s", name="ps_v")
                    for dc in range(DCH):
                        nc.tensor.matmul(
                            ps[:],
                            xt_sb[j][dc][:, c * 128:(c + 1) * 128],
                            w_sb["v"][dc][:],
                            start=(dc == 0), stop=False,
                        )
                    nc.tensor.matmul(
                        ps[:], ones_row[:], bv_sb[:],
                        start=False, stop=True,
                    )
                    nc.vector.tensor_copy(
                        out=V[sc][:].rearrange(
                            "p (h j) -> p h j", j=VSTRIDE)[:, :, 0:64],
                        in_=ps.rearrange("p (h j) -> p h j", j=64),
                    )

                # prologue: V chains first (each gated only on its xT column
                # group, so they run under the DMA), then all Q^T/K^T chains
                for sc in range(SCH):
                    v_chain(sc)
                for mi in range(MCH):
                    for q in range(4):
                        qk_chain("q", bq_sb, QT, mi, q)
                for mi in range(MCH):
                    for q in range(4):
                        qk_chain("k", bk_sb, KT, mi, q)

                # ---- attention: software-pipelined over (head, q-block) ----
                s_tiles = {}  # (block_idx, kc) -> PSUM scores tile
                ctxs = {}     # block_idx -> ctx PSUM tile

                def alloc_ctx(bi):
                    ctxs[bi] = ctxp.tile([128, QW], F32, tag="ctx",
                                         name="ctx_full")

                def emit_scores(bi, kc):
                    h, qb = blocks[bi]
                    mi = h // 2
                    pr = (h % 2) * 64
                    qo = qb * QW
                    t = scp.tile([128, QW], F32, tag="s", name="s_ps")
                    s_tiles[(bi, kc)] = t
                    for q2 in range(QW // 512):
                        nc.tensor.matmul(
                            t[:, q2 * 512:(q2 + 1) * 512],
                            KT[mi][pr:pr + 64, kc * 128:(kc + 1) * 128],
                            QT[mi][pr:pr + 64,
                                   qo + q2 * 512:qo + (q2 + 1) * 512],
                            start=True, stop=True,
                        )

                def emit_tail(ctx_full, recip, h, qo):
                    # round 1/denom to f32r, then broadcast it across
                    # partitions via a K=1 f32r matmul (PSUM dst must start
                    # at partition 0, so it borrows a scores-pool slot)
                    recip_r = rcp.tile([1, QW], F32R, tag="rr", name="recip_r")
                    nc.vector.tensor_copy(out=recip_r[:], in_=recip[:])
                    b_ps = scp.tile([64, QW], F32, tag="s", name="b_ps")
                    for q2 in range(QW // 512):
                        nc.tensor.matmul(
                            b_ps[:, q2 * 512:(q2 + 1) * 512],
                            ones64[:],
                            recip_r[:, q2 * 512:(q2 + 1) * 512],
                            start=True, stop=True,
                        )
                    # TensorTensor may read only one input from PSUM: stage
                    # the broadcast rows through SBUF first
                    bc_sb = obp.tile([64, QW], F32, tag="bc", name="bc_sb",
                                     bufs=1)
                    nc.vector.tensor_copy(out=bc_sb[:], in_=b_ps[:])
                    o_sb = obp.tile([64, QW], F32, tag="o", name="o_sb")
                    nc.vector.tensor_tensor(
                        o_sb[:], ctx_full[0:64, :], bc_sb[:], MULT)
                    nc.sync.dma_start(out[h][:, qo:qo + QW], o_sb[:])

                alloc_ctx(0)
                emit_scores(0, 0)
                emit_scores(0, 1)
                pending = None
                for bi in range(NBLK):
                    h, qb = blocks[bi]
                    qo = qb * QW
                    ctx_full = ctxs.pop(bi)
                    ctx_ps = ctx_full[0:65, :]
                    for kc in range(SCH):
                        # previous block's tail goes here, two iterations in,
                        # so its b_ps matmuls never wait on the reciprocal's
                        # DVE latency in front of this block's PV/scores
                        if kc == 2 and pending is not None:
                            emit_tail(*pending)
                            pending = None
                        pt = ptp.tile([128, QW], F32R, tag="pt", name="pt")
                        nc.scalar.activation(
                            pt[:], s_tiles.pop((bi, kc))[:], EXP,
                            bias=mask_sb[:, kc:kc + 1], scale=0.125,
                        )
                        for q2 in range(QW // 512):
                            nc.tensor.matmul(
                                ctx_ps[:, q2 * 512:(q2 + 1) * 512],
                                V[kc][:, h * VSTRIDE:h * VSTRIDE + 65],
                                pt[:, q2 * 512:(q2 + 1) * 512],
                                start=(kc == 0), stop=(kc == SCH - 1),
                                skip_group_check=True,
                            )
                        # keep the scores pipeline two iterations ahead,
                        # rolling into the next block at this block's end
                        nxt = kc + 2
                        if nxt < SCH:
                            emit_scores(bi, nxt)
                        elif bi + 1 < NBLK:
                            if nxt == SCH:
                                alloc_ctx(bi + 1)
                            emit_scores(bi + 1, nxt - SCH)
                    recip = rcp.tile([1, QW], F32, tag="r", name="recip")
                    nc.vector.reciprocal(recip[:], ctx_ps[64:65, :])
                    pending = (ctx_full, recip, h, qo)
                emit_tail(*pending)

    nc.compile()
    _cached[("nc", reps)] = nc
    return nc


def shard_inputs(hidden_states, attention_mask, Wq, bq, Wk, bk, Wv, bv):
    """Host-side layout prep (no FLOPs): slice + transpose per core."""
    hidden_states = np.asarray(hidden_states, dtype=np.float32)
    attention_mask = np.asarray(attention_mask, dtype=np.float32)
    Wq, Wk, Wv = (np.asarray(w, dtype=np.float32) for w in (Wq, Wk, Wv))
    bq, bk, bv = (np.asarray(b, dtype=np.float32) for b in (bq, bk, bv))
    in_maps = []
    for c in range(N_CORES):
        b_idx, g = divmod(c, 2)
        cols = slice(g * DL, (g + 1) * DL)
        in_maps.append({
            "xT": np.ascontiguousarray(hidden_states[b_idx].T),
            "wq": np.ascontiguousarray(Wq[:, cols]),
            "wk": np.ascontiguousarray(Wk[:, cols]),
            "wv": np.ascontiguousarray(Wv[:, cols]),
            "bq": np.ascontiguousarray(bq[cols].reshape(MCH, 128).T),
            "bk": np.ascontiguousarray(bk[cols].reshape(MCH, 128).T),
            "bv": np.ascontiguousarray(bv[cols].reshape(1, DL)),
            "mask": np.ascontiguousarray(
                attention_mask[b_idx, 0, 0].reshape(SCH, 128).T),
        })
    return in_maps


def assemble_output(results):
    """results: per-core dicts with 'out' [HL, DH, S] -> full [B, S, D]."""
    final = np.empty((B, S, D), dtype=np.float32)
    for b_idx in range(B):
        parts = [results[2 * b_idx + g]["out"] for g in range(2)]  # [6, 64, S]
        ctxT = np.concatenate(parts, axis=0)                       # [12, 64, S]
        final[b_idx] = ctxT.transpose(2, 0, 1).reshape(S, D)
    return final


def kernel(**inputs) -> np.ndarray:
    nc = build_program()
    in_maps = shard_inputs(**inputs)
    res = run_bass_kernel_spmd(nc, in_maps, core_ids=list(range(N_CORES)))
    return assemble_output(res.results)
